# revision 47
# baseline (speedup 1.0000x reference)
"""Multi-Head Latent Attention for Trainium2, sharded over 8 NeuronCores.

Sharding: batch (2) x head-groups (4 of 4 heads each) -> 8 cores.

Key optimizations over the staged baseline:
- MLA weight absorption on host: W_QC = W_DQ @ W_UQ[:, heads] and
  W_QR' = W_DQ @ W_QR[:, heads] fold the query down-projection into the
  up-projections, so the device computes q directly from x and never
  materializes c_Q (which the baseline recomputed redundantly on every
  core of a batch group).
- Fully fused single pass over s-blocks of 512: per block, compute
  c_KV^T, k_rope^T, q_C^T, q_rope^T, k_C^T, v_C, flash attention and
  the W_O partial -- all intermediates SBUF-resident in bf16, no DRAM
  bounce (PE matmul cost is per-output-row regardless of dtype, so
  bf16 halves SBUF/DMA at no PE cost).
- Softmax denominator: exp tiles are accumulated on the Vector engine
  into one [128,512] running sum, needing a single ones-matmul per
  (head, block) instead of one per key tile, and the reciprocal is
  partition-broadcast with a K=1 matmul instead of a DRAM bounce.

Each core's [E,S] partial output is summed across the 4 head-group
cores per batch on host and transposed back.
"""
import numpy as np

import concourse.bass as bass
import concourse.mybir as mybir
import concourse.tile as tile
from concourse import bacc
from concourse.bass_utils import run_bass_kernel_spmd

F32 = mybir.dt.float32
F32R = mybir.dt.float32r
BF = mybir.dt.bfloat16
Exp = mybir.ActivationFunctionType.Exp
Copy = mybir.ActivationFunctionType.Copy
Recip = mybir.ActivationFunctionType.Reciprocal
Mult = mybir.AluOpType.mult

B, S, E = 2, 2048, 2048
H = 16
DH = 128
LOW = 512
R = 64
BASE = 10000.0
HPG = 4               # heads per group (per core)
GCOL = HPG * DH       # 512 columns of this group's heads
P = 128
KE = E // P           # 16 k-tiles over E
KL = LOW // P         # 4 k-tiles over LOW
ST = S // P           # 16 seq tiles of 128
SBN = S // 512        # 4 seq blocks of 512
NEG = -3.0e38
SCALE = 1.0 / float(np.sqrt(DH + R))

_CACHE = {}


def _lhsT_layout(w):
    """[K, M] -> [MT, 128, KT, 128] so slice [mo] is an SBUF tile
    [128p, KT, 128m] with element [p, ko, m] = w[ko*128+p, mo*128+m]."""
    K, M = w.shape
    return np.ascontiguousarray(
        w.reshape(K // P, P, M // P, P).transpose(2, 1, 0, 3))


def _rhs_layout(w):
    """[K, N] -> [128, KT, N]: element [p, ko, n] = w[ko*128+p, n]."""
    K, N = w.shape
    return np.ascontiguousarray(w.reshape(K // P, P, N).transpose(1, 0, 2))


def _rope_perm_cols(w, rope_dim=R):
    """Permute each rope_dim-column block to [evens, odds] order."""
    K, M = w.shape
    nh = M // rope_dim
    w = w.reshape(K, nh, rope_dim)
    perm = np.concatenate([np.arange(0, rope_dim, 2), np.arange(1, rope_dim, 2)])
    return np.ascontiguousarray(w[:, :, perm].reshape(K, M))


def build_nc():
    nc = bacc.Bacc("TRN2", target_bir_lowering=False, debug=False, num_devices=8)

    xTd = nc.dram_tensor("xTd", [P, KE, S], BF, kind="ExternalInput")
    wQC = nc.dram_tensor("wQC", [HPG, P, KE, P], BF, kind="ExternalInput")
    wQR = nc.dram_tensor("wQR", [2, P, KE, P], BF, kind="ExternalInput")
    wDKV = nc.dram_tensor("wDKV", [KL, P, KE, P], BF, kind="ExternalInput")
    wUK = nc.dram_tensor("wUK", [HPG, P, KL, P], BF, kind="ExternalInput")
    wUV = nc.dram_tensor("wUV", [P, KL, GCOL], BF, kind="ExternalInput")
    wKR = nc.dram_tensor("wKR", [P, KE, R], BF, kind="ExternalInput")
    wO = nc.dram_tensor("wO", [P, HPG, E], BF, kind="ExternalInput")
    cosq = nc.dram_tensor("cosq", [P, S], F32, kind="ExternalInput")   # [cos;cos]x2
    sinq = nc.dram_tensor("sinq", [P, S], F32, kind="ExternalInput")   # [-sin;sin]x2
    maskin = nc.dram_tensor("maskin", [P, 4, 512], BF, kind="ExternalInput")
    ones_in = nc.dram_tensor("ones_in", [P, P], BF, kind="ExternalInput")

    outT = nc.dram_tensor("outT", [E, S], BF, kind="ExternalOutput")

    with tile.TileContext(nc) as tc:
        with (
            tc.tile_pool(name="persist", bufs=1) as persist,
            tc.tile_pool(name="xp", bufs=12) as xp,
            tc.tile_pool(name="ckvp", bufs=2) as ckvp,
            tc.tile_pool(name="qcp", bufs=2) as qcp,
            tc.tile_pool(name="qrp", bufs=2) as qrp,
            tc.tile_pool(name="ropetmp", bufs=2) as ropetmp,
            tc.tile_pool(name="etp", bufs=4) as etp,
            tc.tile_pool(name="esp", bufs=2) as esp,
            tc.tile_pool(name="btp", bufs=2) as btp,
            tc.tile_pool(name="aop", bufs=2) as aop,
            tc.tile_pool(name="oout", bufs=4) as oout,
            tc.tile_pool(name="ps_mm", bufs=2, space="PSUM") as ps_mm,
            tc.tile_pool(name="ps_s", bufs=3, space="PSUM") as ps_s,
            tc.tile_pool(name="ps_o", bufs=2, space="PSUM") as ps_o,
            tc.tile_pool(name="ps_bc", bufs=1, space="PSUM") as ps_bc,
        ):
            # ---- persistent tiles -------------------------------------
            kropeT = persist.tile([P, S], BF, tag="kropeT")
            kCT = persist.tile([P, HPG, S], BF, tag="kCT")
            vC = persist.tile([P, ST, GCOL], BF, tag="vC")

            wdma = nc.gpsimd.dma_start
            t_wdkv = [persist.tile([P, KE, P], BF, tag=f"wdkv{m}",
                                   name=f"wdkv{m}") for m in range(KL)]
            wdma(out=t_wdkv[0][:, 0:4, :], in_=wDKV[0, :, 0:4, :])
            wdma(out=t_wdkv[0][:, 4:KE, :], in_=wDKV[0, :, 4:KE, :])
            for m in range(1, KL):
                wdma(out=t_wdkv[m], in_=wDKV[m])
            t_wkr = persist.tile([P, KE, R], BF, tag="wkr")
            wdma(out=t_wkr, in_=wKR[:, :, :])
            t_wqc = [persist.tile([P, KE, P], BF, tag=f"wqc{m}",
                                  name=f"wqc{m}") for m in range(HPG)]
            for m in range(HPG):
                wdma(out=t_wqc[m], in_=wQC[m])
            t_wqr = [persist.tile([P, KE, P], BF, tag=f"wqr{m}",
                                  name=f"wqr{m}") for m in range(2)]
            for m in range(2):
                wdma(out=t_wqr[m], in_=wQR[m])
            t_wuk = [persist.tile([P, KL, P], BF, tag=f"wuk{m}",
                                  name=f"wuk{m}") for m in range(HPG)]
            for m in range(HPG):
                wdma(out=t_wuk[m], in_=wUK[m])
            t_wuv = persist.tile([P, KL, GCOL], BF, tag="wuv")
            wdma(out=t_wuv, in_=wUV[:, :, :])
            t_wo = persist.tile([P, HPG, E], BF, tag="wo")
            wdma(out=t_wo, in_=wO[:, :, :])
            t_cos = persist.tile([P, S], F32, tag="cos")
            wdma(out=t_cos, in_=cosq[:, :])
            t_sin = persist.tile([P, S], F32, tag="sin")
            wdma(out=t_sin, in_=sinq[:, :])
            t_mask = persist.tile([P, 4, 512], BF, tag="mask")
            wdma(out=t_mask, in_=maskin[:, :, :])
            t_ones = persist.tile([P, P], BF, tag="ones")
            wdma(out=t_ones, in_=ones_in[:, :])

            def rope_from_psum(psum, base, scol, dst):
                """dst[...] (64 x 512, bf16) = rope(psum[base:base+64]).

                psum rows [base:base+32]=x1, [base+32:base+64]=x2 (host
                permuted weight cols). dst = aln*[c;c] + swp*[-s;s]."""
                sl = slice(scol, scol + 512)
                aln = ropetmp.tile([R, 512], F32, tag="aln")
                nc.vector.tensor_copy(out=aln, in_=psum[base:base + R, :])
                swp = ropetmp.tile([R, 512], F32, tag="swp")
                nc.vector.tensor_copy(out=swp[0:32, :], in_=aln[32:R, :])
                nc.vector.tensor_copy(out=swp[32:R, :], in_=aln[0:32, :])
                nc.vector.tensor_mul(out=aln, in0=aln, in1=t_cos[0:R, sl])
                nc.vector.tensor_mul(out=swp, in0=swp, in1=t_sin[0:R, sl])
                nc.vector.tensor_add(out=dst, in0=aln, in1=swp)

            def rope2_from_psum(psum, scol, dst_a, dst_b):
                """Rope of two stacked heads in one [128,512] op set.

                psum rows [0:32]=x1_a, [32:64]=x2_a, [64:96]=x1_b,
                [96:128]=x2_b. dst_a/dst_b are [64, 512] bf16 slices at
                base partition 0 (DVE lanes are fixed, so only the final
                adds are split per head)."""
                sl = slice(scol, scol + 512)
                aln = ropetmp.tile([P, 512], F32, tag="aln2")
                nc.vector.tensor_copy(out=aln, in_=psum)
                swp = ropetmp.tile([P, 512], F32, tag="swp2")
                nc.vector.tensor_copy(out=swp[0:32, :], in_=aln[32:64, :])
                nc.vector.tensor_copy(out=swp[32:64, :], in_=aln[0:32, :])
                nc.vector.tensor_copy(out=swp[64:96, :], in_=aln[96:128, :])
                nc.vector.tensor_copy(out=swp[96:128, :], in_=aln[64:96, :])
                nc.vector.tensor_mul(out=aln, in0=aln, in1=t_cos[:, sl])
                nc.vector.tensor_mul(out=swp, in0=swp, in1=t_sin[:, sl])
                nc.vector.tensor_add(out=dst_a, in0=aln[0:R, :], in1=swp[0:R, :])
                nc.vector.tensor_add(out=dst_b, in0=aln[R:P, :], in1=swp[R:P, :])

            KH = KE // 2

            def projections(sb):
                """Everything derivable from x[sb]: ckv, krope, qc, qr,
                kc, vc. Returns (qc, qr) for the attention pass."""
                ssl = slice(sb * 512, (sb + 1) * 512)
                xq = []
                for i in range(8):
                    t = xp.tile([P, 2, 512], BF, tag="xt")
                    nc.sync.dma_start(out=t, in_=xTd[:, 2 * i:2 * i + 2, ssl])
                    xq.append(t)

                def xk(k):
                    return xq[k // 2][:, k % 2, :]

                # ---- c_KV^T for this block ----------------------------
                ckvt = ckvp.tile([P, KL, 512], BF, tag="ckvt")
                for mo in range(KL):
                    ps = ps_mm.tile([P, 512], F32, tag="p")
                    for k in range(KE):
                        nc.tensor.matmul(ps, t_wdkv[mo][:, k, :], xk(k),
                                         start=(k == 0), stop=(k == KE - 1))
                    nc.scalar.activation(out=ckvt[:, mo, :], in_=ps, func=Copy)

                # ---- k_rope^T -----------------------------------------
                ps = ps_mm.tile([P, 512], F32, tag="p")
                for k in range(KE):
                    nc.tensor.matmul(ps[0:R, :], t_wkr[:, k, :], xk(k),
                                     start=(k == 0), stop=(k == KE - 1))
                rope_from_psum(ps, 0, sb * 512, kropeT[0:R, ssl])
                nc.sync.dma_start(out=kropeT[R:P, ssl], in_=kropeT[0:R, ssl])

                # ---- q_C^T / q_rope^T, with the short k_C/v_C chains
                # interleaved after long chains so their psum-copy
                # latency hides under the next long chain --------------
                qc = qcp.tile([P, HPG, 512], BF, tag="qc")
                qr = qrp.tile([P, 2, 512], BF, tag="qr")

                def qc_chain(h):
                    ps = ps_mm.tile([P, 512], F32, tag="p")
                    for k in range(KE):
                        nc.tensor.matmul(ps, t_wqc[h][:, k, :], xk(k),
                                         start=(k == 0), stop=(k == KE - 1))
                    nc.scalar.activation(out=qc[:, h, :], in_=ps, func=Copy)

                def qr_chain(j):
                    ps = ps_mm.tile([P, 512], F32, tag="p")
                    for k in range(KE):
                        nc.tensor.matmul(ps, t_wqr[j][:, k, :], xk(k),
                                         start=(k == 0), stop=(k == KE - 1))
                    rope2_from_psum(ps, sb * 512, qr[0:R, j, :],
                                    qr[R:P, j, :])

                def kc_chain(h):
                    ps = ps_mm.tile([P, 512], F32, tag="p")
                    for k in range(KL):
                        nc.tensor.matmul(ps, t_wuk[h][:, k, :], ckvt[:, k, :],
                                         start=(k == 0), stop=(k == KL - 1))
                    nc.scalar.activation(out=kCT[:, h, ssl], in_=ps, func=Copy)

                def vc_chain(loc):
                    ps = ps_mm.tile([P, GCOL], F32, tag="p")
                    for k in range(KL):
                        nc.tensor.matmul(ps, ckvt[:, k, loc * P:(loc + 1) * P],
                                         t_wuv[:, k, :],
                                         start=(k == 0), stop=(k == KL - 1))
                    nc.scalar.activation(out=vC[:, sb * 4 + loc, :], in_=ps,
                                         func=Copy)

                qc_chain(0); kc_chain(0)
                qc_chain(1); kc_chain(1)
                qc_chain(2); kc_chain(2)
                qc_chain(3); kc_chain(3)
                qr_chain(0); vc_chain(0); vc_chain(1)
                vc_chain(2); vc_chain(3); qr_chain(1)
                return qc, qr

            def attention(sb, qc, qr):
                # ---- flash attention over t <= s ----------------------
                aoT = aop.tile([P, HPG, 512], BF, tag="aoT")
                T = 4 * (sb + 1)
                for h in range(HPG):
                    ps_ot = ps_o.tile([P, 512], F32, tag="p")
                    expsum = esp.tile([P, 512], BF, tag="es")
                    for tt in range(T):
                        tsl = slice(tt * P, (tt + 1) * P)
                        # diagonal tiles r=tt-4sb: cols < 128r are fully
                        # masked (t > s), skip computing them entirely
                        r = tt - 4 * sb
                        c0 = max(0, r) * P
                        csl = slice(c0, 512)
                        ps_st = ps_s.tile([P, 512], F32, tag="p")
                        nc.tensor.matmul(ps_st[:, csl], kCT[:, h, tsl],
                                         qc[:, h, csl],
                                         start=True, stop=False)
                        po = R * (h % 2)
                        nc.tensor.matmul(ps_st[:, csl],
                                         kropeT[po:po + R, tsl],
                                         qr[po:po + R, h // 2, csl],
                                         start=False, stop=True)
                        if r >= 0:
                            msl = slice(c0, c0 + P)
                            nc.vector.tensor_add(out=ps_st[:, msl],
                                                 in0=ps_st[:, msl],
                                                 in1=t_mask[:, r, msl])
                        expT = etp.tile([P, 512], BF, tag="expT")
                        nc.scalar.activation(out=expT[:, csl], in_=ps_st[:, csl],
                                             func=Exp)
                        if tt == 0:
                            nc.vector.tensor_copy(out=expsum, in_=expT)
                        else:
                            nc.vector.tensor_add(out=expsum[:, csl],
                                                 in0=expsum[:, csl],
                                                 in1=expT[:, csl])
                        nc.tensor.matmul(ps_ot[:, csl],
                                         vC[:, tt, h * DH:(h + 1) * DH],
                                         expT[:, csl],
                                         start=(tt == 0), stop=(tt == T - 1))
                    # ones[128,128]^T @ expsum = column sums broadcast
                    # across all partitions in one matmul
                    bc = ps_bc.tile([P, 512], F32, tag="bc")
                    nc.tensor.matmul(bc, t_ones, expsum, start=True, stop=True)
                    bcs = btp.tile([P, 512], F32, tag="bcs")
                    nc.vector.reciprocal(out=bcs, in_=bc)
                    nc.vector.tensor_tensor(aoT[:, h, :], ps_ot, bcs, Mult)
                return aoT

            def wo_block(sb, aoT):
                # ---- W_O partial for this block -----------------------
                ssl = slice(sb * 512, (sb + 1) * 512)
                for mo in range(KE):
                    ps = ps_mm.tile([P, 512], F32, tag="p")
                    for hh in range(HPG):
                        nc.tensor.matmul(ps,
                                         t_wo[:, hh, mo * P:(mo + 1) * P],
                                         aoT[:, hh, :],
                                         start=(hh == 0), stop=(hh == HPG - 1))
                    ot = oout.tile([P, 512], BF, tag="oo")
                    if mo % 2 == 0:
                        nc.scalar.activation(out=ot, in_=ps, func=Copy)
                    else:
                        nc.vector.tensor_copy(out=ot, in_=ps)
                    nc.gpsimd.dma_start(out=outT[mo * P:(mo + 1) * P, ssl], in_=ot)

            # Software pipeline: the sb+1 projections are issued between
            # attention(sb) and wo(sb) so the dense independent matmul
            # stream absorbs the softmax-normalize tail latency.
            q_sb = projections(0)
            for sb in range(SBN):
                aoT = attention(sb, *q_sb)
                if sb + 1 < SBN:
                    q_sb = projections(sb + 1)
                wo_block(sb, aoT)

    nc.compile()
    return nc


def _host_inputs(inputs):
    """Per-core input maps (host-side sharding + weight pre-tiling)."""
    npbf = mybir.dt.np(BF)
    f32 = np.float32
    x = inputs["x"].astype(f32)
    W_DQ, W_UQ, W_QR = (inputs[k].astype(f32) for k in ("W_DQ", "W_UQ", "W_QR"))
    W_DKV, W_UK, W_KR = (inputs[k].astype(f32) for k in ("W_DKV", "W_UK", "W_KR"))
    W_UV, W_O = (inputs[k].astype(f32) for k in ("W_UV", "W_O"))

    # MLA absorption: fold the query down-projection into the up-projections.
    W_QCa = (W_DQ @ W_UQ) * SCALE          # [E, E]
    W_QRa = (W_DQ @ W_QR) * SCALE          # [E, R*H]

    # shared across cores
    wDKV_t = _lhsT_layout(W_DKV).astype(npbf)
    wKR_t = _rhs_layout(_rope_perm_cols(W_KR)).astype(npbf)  # [128,KE,64]
    half = R // 2
    freqs = BASE ** (-np.arange(half, dtype=np.float64) / half)
    theta = np.arange(S, dtype=np.float64)[None, :] * freqs[:, None]   # [32, S]
    cos2 = np.tile(np.concatenate([np.cos(theta), np.cos(theta)], 0),
                   (2, 1)).astype(f32)
    sinpm = np.tile(np.concatenate([-np.sin(theta), np.sin(theta)], 0),
                    (2, 1)).astype(f32)
    p = np.arange(P)[:, None, None]
    rr = np.arange(4)[None, :, None]
    f = np.arange(512)[None, None, :]
    maskadd = np.where(p <= f - P * rr, 0.0, NEG).astype(npbf)
    ones = np.ones((P, P), npbf)

    in_maps = []
    for c in range(8):
        b, g = divmod(c, 4)
        cs, ce = g * GCOL, (g + 1) * GCOL          # head cols of this group
        wQC_g = _lhsT_layout(W_QCa[:, cs:ce]).astype(npbf)
        qr = W_QRa[:, g * HPG * R:(g + 1) * HPG * R]
        wQR_g = _lhsT_layout(_rope_perm_cols(qr)).astype(npbf)
        wUK_g = _lhsT_layout(W_UK[:, cs:ce]).astype(npbf)
        wUV_g = _rhs_layout(W_UV[:, cs:ce]).astype(npbf)
        wO_g = _rhs_layout(W_O[cs:ce, :]).astype(npbf)       # [128, 4, E]
        xb = np.ascontiguousarray(
            x[b].T.reshape(KE, P, S).transpose(1, 0, 2)).astype(npbf)
        in_maps.append({
            "xTd": xb,
            "wQC": wQC_g, "wQR": wQR_g, "wDKV": wDKV_t,
            "wUK": wUK_g, "wUV": wUV_g, "wKR": wKR_t, "wO": wO_g,
            "cosq": cos2, "sinq": sinpm, "maskin": maskadd,
            "ones_in": ones,
        })
    return in_maps


def _assemble(results):
    out = np.empty((B, S, E), np.float32)
    for b in range(B):
        acc = results[4 * b]["outT"].astype(np.float32).copy()
        for g in range(1, 4):
            acc += results[4 * b + g]["outT"].astype(np.float32)
        out[b] = acc.T
    return out


def kernel(**inputs):
    inputs = {k: np.asarray(v) for k, v in inputs.items()}
    if "nc" not in _CACHE:
        _CACHE["nc"] = build_nc()
    nc = _CACHE["nc"]
    in_maps = _host_inputs(inputs)
    res = run_bass_kernel_spmd(nc, in_maps, core_ids=list(range(8)))
    return _assemble(res.results)
